# revision 33
# baseline (speedup 1.0000x reference)
"""Supervised-contrastive loss on 8 TRN2 NeuronCores — v3 (symmetric half).

S = fn@fnT is symmetric, so only ~half the E matrix is computed:
16 row-strips of 256; strip s covers wrapped column range
[s*256, s*256+W) mod 4096 with W = 2304 (even-parity strips, includes the
distance-8 block) or 2048 (odd strips).  Every unordered (i,j) pair is then
computed exactly once.  Core c owns strips c and 15-c (opposite parity ->
every core gets one 2304 strip and one 2048 strip: uniform SPMD program).

Per strip the device produces:
  * row sums of E over the computed range (ACT exp accumulator),
  * masked window sum over local cols [0, 384) (positives of the strip's
    rows minus any left-straddling class, corrected on host),
  * E diagonal,
  * column sums of E over local cols [256, W) (PE ones-matmul) -- these are
    the mirrored row-sum contributions for OTHER strips' rows; the host
    scatter-adds them (rsE_i = own rowsum + sum of colsum partials at i).
Host finishes with the validated ln-free approximation (see kernel.py).
"""

import numpy as np
import ml_dtypes

TAU = 0.1
N, D = 4096, 512
NCORES = 8
NSTRIP = 16
SROWS = 256                  # rows per strip
KK = 2
SCALE = 16.0
WIN = 384                    # masked-window width per strip (max class <= 128)
WE, WO = 2304, 2048          # computed widths (even / odd parity strips)
ESPLIT = 1536                # fnE DMA split point (matmul-slice aligned)
CHUNKS_E = ((0, 1024), (1024, 2048), (2048, 2304))
CHUNKS_O = ((0, 1024), (1024, 2048))
# csum segment table: (sslot, c0, c1, out_offset) for cols [max(c0,256), c1)
CSEG = []
_off = 0
for _ss, _chunks in ((0, CHUNKS_E), (1, CHUNKS_O)):
    for _c0, _c1 in _chunks:
        _s0 = max(_c0, 256)
        CSEG.append((_ss, _c0, _c1, _s0, _off))
        _off += _c1 - _s0
CSUM_W = _off                # 3840

_CACHE = {}


def _strips(c):
    # slotE (width 2304, covers block-distance 0..8) = strips 0..7;
    # slotO (width 2048, distance 0..7) = strips 8..15.  A distance-8 pair
    # (si, si+8) is then covered exactly once, by the low strip.
    return (c, NSTRIP - 1 - c)


def _build_nc():
    import concourse.tile as tile
    import concourse.mybir as mybir
    from concourse import bacc

    dt = mybir.dt
    AF = mybir.ActivationFunctionType
    ALU = mybir.AluOpType
    AX = mybir.AxisListType
    PM = mybir.MatmulPerfMode

    nc = bacc.Bacc(None)
    # fnE split per kk at the first chunk boundary: fully-contiguous DMAs
    fnea = [nc.declare_dram_parameter(f"fnea{k}", [128, 2, ESPLIT], dt.float8e4,
                                      isOutput=False) for k in range(KK)]
    fneb = [nc.declare_dram_parameter(f"fneb{k}", [128, 2, WE - ESPLIT], dt.float8e4,
                                      isOutput=False) for k in range(KK)]
    fno = nc.declare_dram_parameter("fno", [128, 4, WO], dt.float8e4, isOutput=False)
    lhsd = nc.declare_dram_parameter("lhsd", [128, 8, SROWS], dt.float8e4, isOutput=False)
    tbs = nc.declare_dram_parameter("tbs", [128, 2, WIN], dt.bfloat16, isOutput=False)
    tcols = nc.declare_dram_parameter("tcols", [128, 4], dt.float32, isOutput=False)
    iot = nc.declare_dram_parameter("iot", [128, 128], dt.bfloat16, isOutput=False)
    prow = nc.declare_dram_parameter("prow", [128, 1], dt.float32, isOutput=False)
    onesd = nc.declare_dram_parameter("onesd", [128, 1], dt.bfloat16, isOutput=False)
    # res: rse[slot 0..3] | rsem[4..7] | eii[8..11], slot = sslot*2 + rt
    res_out = nc.declare_dram_parameter("res_out", [128, 12], dt.float32, isOutput=True)
    csum_out = nc.declare_dram_parameter("csum_out", [1, CSUM_W], dt.float32, isOutput=True)

    with tile.TileContext(nc) as tc:
        with (
            tc.tile_pool(name="persist", bufs=1) as persist,
            tc.tile_pool(name="psum", bufs=2, space="PSUM") as psum,
            tc.tile_pool(name="cspsum", bufs=2, space="PSUM") as cspsum,
            tc.tile_pool(name="ebuf", bufs=8) as ebuf,
            tc.tile_pool(name="scr", bufs=2) as scr,
            tc.tile_pool(name="acc", bufs=1) as accp,
            tc.tile_pool(name="outp", bufs=1) as outp,
        ):
            fnO_sb = [None, None]
            lhs_sb = [[None, None], [None, None]]
            with tc.high_priority():
                fnEa_sb = [None, None]
                fnEb_sb = [None, None]
                for kk in range(KK):
                    fnEa_sb[kk] = persist.tile([128, 2, ESPLIT], dt.float8e4,
                                               name=f"fnEa_{kk}", tag=f"fnEa_{kk}")
                    fnEb_sb[kk] = persist.tile([128, 2, WE - ESPLIT], dt.float8e4,
                                               name=f"fnEb_{kk}", tag=f"fnEb_{kk}")
                    fnO_sb[kk] = persist.tile([128, 2, WO], dt.float8e4,
                                              name=f"fnO_{kk}", tag=f"fnO_{kk}")
                    for ss in range(2):
                        lhs_sb[ss][kk] = persist.tile(
                            [128, 2, SROWS], dt.float8e4,
                            name=f"lhs_{ss}_{kk}", tag=f"lhs_{ss}_{kk}")
                # whole-plane DMAs (>=4KB contiguous lines).  sync and scalar
                # each carry one fnE half so chunk-0 data lands earliest;
                # scalar's single issue finishes long before its first exp.
                nc.sync.dma_start(fnEa_sb[0][:, :, :], fnea[0][:, :, :])
                nc.scalar.dma_start(fnEa_sb[1][:, :, :], fnea[1][:, :, :])
                nc.sync.dma_start(fnEb_sb[0][:, :, :], fneb[0][:, :, :])
                nc.scalar.dma_start(fnEb_sb[1][:, :, :], fneb[1][:, :, :])
                for ss in range(2):
                    for kk in range(KK):
                        nc.gpsimd.dma_start(
                            lhs_sb[ss][kk][:, :, :],
                            lhsd[:, (ss * 2 + kk) * 2:(ss * 2 + kk) * 2 + 2, :])
                tbs_sb = persist.tile([128, 2, WIN], dt.bfloat16, tag="tbs")
                nc.gpsimd.dma_start(tbs_sb[:, :, :], tbs[:, :, :])
                tcols_sb = persist.tile([128, 4], dt.float32, tag="tcols")
                nc.gpsimd.dma_start(tcols_sb[:], tcols[:])
                iot_sb = persist.tile([128, 128], dt.bfloat16, tag="iot")
                nc.gpsimd.dma_start(iot_sb[:], iot[:])
                prow_sb = persist.tile([128, 1], dt.float32, tag="prow")
                nc.gpsimd.dma_start(prow_sb[:], prow[:])
                ones_sb = persist.tile([128, 1], dt.bfloat16, tag="ones")
                nc.gpsimd.dma_start(ones_sb[:], onesd[:])
                nc.scalar.dma_start(fnO_sb[0][:, :, :], fno[:, 0:2, :])
                nc.gpsimd.dma_start(fnO_sb[1][:, :, :], fno[:, 2:4, :])

            # PE warm-up: dummy DR matmuls on zeroed tiles during the DMA
            # wait flip the HAM clock gate to 2.4 GHz before the real GEMM
            wz = persist.tile([128, 2, 512], dt.float8e4, tag="wz")
            nc.vector.memset(wz[:, :, :], 0)
            wl = persist.tile([128, 2, 128], dt.float8e4, tag="wl")
            nc.vector.memset(wl[:, :, :], 0)
            Sw = psum.tile([128, 1024], dt.float32, tag="S")
            for _ in range(8):
                nc.tensor.matmul(Sw[:, 0:512], wl[:, :, :], wz[:, :, :],
                                 start=True, stop=True, perf_mode=PM.DoubleRow)

            res_sb = outp.tile([128, 12], dt.float32, tag="res")
            csum_sb = outp.tile([1, CSUM_W], dt.float32, tag="csum")
            rse2 = [[accp.tile([128, 3], dt.float32, name=f"rse2_{ss}_{rt}",
                               tag=f"rse2_{ss}_{rt}") for rt in range(2)]
                    for ss in range(2)]

            chunk_list = ([(0, ci, c0, c1) for ci, (c0, c1) in enumerate(CHUNKS_E)]
                          + [(1, ci, c0, c1) for ci, (c0, c1) in enumerate(CHUNKS_O)])
            pending = []   # (appended_at_j, Es, s0, c0, c1, off)

            def flush_cs(job):
                _, Es, s0, c0, c1, off = job
                cs = cspsum.tile([1, 1024], dt.float32, name="cs", tag="cs")
                cw = c1 - s0
                for rt in range(2):
                    for p0 in range(0, cw, 512):
                        p1 = min(p0 + 512, cw)
                        nc.tensor.matmul(
                            cs[0:1, p0:p1], ones_sb[:, 0:1],
                            Es[rt][:, s0 - c0 + p0:s0 - c0 + p1],
                            start=(rt == 0), stop=(rt == 1),
                        )
                nc.vector.tensor_copy(
                    csum_sb[0:1, off:off + cw], cs[0:1, 0:cw])

            for j, (ss, ci, c0, c1) in enumerate(chunk_list):
                w = c1 - c0
                s0 = max(c0, 256)

                def _rhs(kk, nb, nw):
                    if ss == 1:
                        return fnO_sb[kk][:, :, c0 + nb:c0 + nb + nw]
                    if c0 + nb < ESPLIT:
                        return fnEa_sb[kk][:, :, c0 + nb:c0 + nb + nw]
                    return fnEb_sb[kk][:, :, c0 + nb - ESPLIT:
                                       c0 + nb - ESPLIT + nw]

                Ss = [psum.tile([128, 1024], dt.float32, name=f"S{rt}", tag="S")
                      for rt in range(2)]
                # kk-outer on the first chunk: both row-tiles' kk0 passes run
                # first, giving the slower queue's kk1 data time to land
                order = ([(kk, rt) for kk in range(KK) for rt in range(2)]
                         if j == 0 else
                         [(kk, rt) for rt in range(2) for kk in range(KK)])
                for kk, rt in order:
                    for nb in range(0, w, 512):
                        nw = min(512, w - nb)
                        nc.tensor.matmul(
                            Ss[rt][:, nb:nb + nw],
                            lhs_sb[ss][kk][:, :, rt * 128:(rt + 1) * 128],
                            _rhs(kk, nb, nw),
                            start=(kk == 0), stop=(kk == KK - 1),
                            perf_mode=PM.DoubleRow,
                        )
                Es = []
                for rt in range(2):
                    E = ebuf.tile([128, 1024], dt.bfloat16, tag="E")
                    nc.scalar.activation(
                        E[:, 0:w], Ss[rt][:, 0:w], AF.Exp,
                        scale=1.0 / (TAU * SCALE * SCALE),
                        accum_out=rse2[ss][rt][:, ci:ci + 1],
                    )
                    Es.append(E)
                    slot = ss * 2 + rt
                    if ci == 0:
                        em_scr = scr.tile([128, WIN], dt.bfloat16, tag="em_scr")
                        nc.vector.scalar_tensor_tensor(
                            em_scr[:], tbs_sb[:, ss, :],
                            tcols_sb[:, slot:slot + 1], E[:, 0:WIN],
                            ALU.is_equal, ALU.mult,
                            accum_out=res_sb[:, 4 + slot:5 + slot],
                        )
                        d_scr = scr.tile([128, 128], dt.bfloat16, tag="d_scr")
                        nc.vector.scalar_tensor_tensor(
                            d_scr[:], iot_sb[:], prow_sb[:],
                            E[:, rt * 128:(rt + 1) * 128],
                            ALU.is_equal, ALU.mult,
                            accum_out=res_sb[:, 8 + slot:9 + slot],
                        )
                if s0 < c1:
                    pending.append((j, Es, s0, c0, c1, CSEG[j][4]))
                # flush colsum jobs two chunks behind: their exp has long
                # finished, so the PE never waits on ACT
                while pending and pending[0][0] <= j - 1:
                    flush_cs(pending.pop(0))
                if ss == 1 and ci == len(CHUNKS_O) - 1:
                    for rt in range(2):
                        nc.vector.tensor_reduce(
                            res_sb[:, 0 * 2 + rt:0 * 2 + rt + 1],
                            rse2[0][rt][:, 0:len(CHUNKS_E)], AX.X, ALU.add)
            while pending:
                flush_cs(pending.pop(0))
            for rt in range(2):
                nc.vector.tensor_reduce(
                    res_sb[:, 1 * 2 + rt:1 * 2 + rt + 1],
                    rse2[1][rt][:, 0:len(CHUNKS_O)], AX.X, ALU.add)

            nc.sync.dma_start(res_out[:], res_sb[:])
            nc.sync.dma_start(csum_out[:], csum_sb[:])

    nc.finalize()
    return nc


def _get_nc():
    if "nc" not in _CACHE:
        _CACHE["nc"] = _build_nc()
    return _CACHE["nc"]


def _host_prep(features, targets):
    f8 = ml_dtypes.float8_e4m3
    bf16 = ml_dtypes.bfloat16
    f = np.asarray(features, np.float32)
    t = np.asarray(targets).astype(np.int64)
    idx = np.argsort(t, kind="stable")
    ts = t[idx]
    assert np.bincount(ts).max() <= 128, "class size exceeds mask window"
    rnorm = 1.0 / np.sqrt((f.astype(np.float64) ** 2).sum(1))
    fn = (f * rnorm[:, None].astype(np.float32)).astype(np.float32)
    fns = fn[idx]
    q = (fns * SCALE).astype(f8)
    qT = np.ascontiguousarray(q.T)                   # [D, N]
    tsb = ts.astype(np.float32)

    iot = np.ascontiguousarray(np.broadcast_to(
        np.arange(128, dtype=np.float32).astype(bf16)[None, :], (128, 128)))
    prow = np.arange(128, dtype=np.float32).reshape(128, 1)
    ones = np.ones((128, 1), bf16)

    def pack(cols, nplane):                          # -> [128, nplane*2? ...]
        # planes (kk, pl): row kk*256+pl*128+p
        out = np.empty((128, 4, len(cols)), f8)
        m = qT[:, cols]
        for kk in range(KK):
            for pl in range(2):
                r0 = kk * 256 + pl * 128
                out[:, kk * 2 + pl, :] = m[r0:r0 + 128, :]
        return out

    in_maps = []
    for c in range(NCORES):
        sE, sO = _strips(c)
        colsE = (sE * SROWS + np.arange(WE)) % N
        colsO = (sO * SROWS + np.arange(WO)) % N
        lhsd = np.empty((128, 8, SROWS), f8)
        lhsd[:, 0:4, :] = pack(np.arange(sE * SROWS, (sE + 1) * SROWS), 4)
        lhsd[:, 4:8, :] = pack(np.arange(sO * SROWS, (sO + 1) * SROWS), 4)
        tbs = np.empty((128, 2, WIN), bf16)
        tbs[:, 0, :] = np.broadcast_to(
            tsb[(sE * SROWS + np.arange(WIN)) % N].astype(bf16)[None, :], (128, WIN))
        tbs[:, 1, :] = np.broadcast_to(
            tsb[(sO * SROWS + np.arange(WIN)) % N].astype(bf16)[None, :], (128, WIN))
        tcols = np.empty((128, 4), np.float32)
        for ss, s in ((0, sE), (1, sO)):
            for rt in range(2):
                tcols[:, ss * 2 + rt] = tsb[s * SROWS + rt * 128 + np.arange(128)]
        pe = pack(colsE, 4)
        in_maps.append({
            "fnea0": np.ascontiguousarray(pe[:, 0:2, 0:ESPLIT]),
            "fnea1": np.ascontiguousarray(pe[:, 2:4, 0:ESPLIT]),
            "fneb0": np.ascontiguousarray(pe[:, 0:2, ESPLIT:]),
            "fneb1": np.ascontiguousarray(pe[:, 2:4, ESPLIT:]),
            "fno": pack(colsO, 4),
            "lhsd": lhsd,
            "tbs": tbs,
            "tcols": tcols,
            "iot": iot,
            "prow": prow,
            "onesd": ones,
        })
    return fns, ts, q, in_maps


def _host_post(fns, ts, q, results):
    rse = np.zeros(N, np.float64)
    rsem = np.zeros(N, np.float64)
    eii = np.zeros(N, np.float64)
    for c, out in enumerate(results):
        res = np.asarray(out["res_out"], np.float64)      # [128, 12]
        csum = np.asarray(out["csum_out"], np.float64)[0]  # [CSUM_W]
        sE, sO = _strips(c)
        for ss, s in ((0, sE), (1, sO)):
            for rt in range(2):
                slot = ss * 2 + rt
                rows = s * SROWS + rt * 128 + np.arange(128)
                rse[rows] += res[:, slot]
                rsem[rows] = res[:, 4 + slot]
                eii[rows] = res[:, 8 + slot]
        for ss, c0, c1, s0, off in CSEG:
            s = sE if ss == 0 else sO
            gcols = (s * SROWS + np.arange(s0, c1)) % N
            rse[gcols] += csum[off:off + (c1 - s0)]

    # host corrections: boundary-straddling classes (left side of each strip)
    qf = q.astype(np.float32)
    for s in range(1, NSTRIP):
        b = s * SROWS
        cls = ts[b]
        if ts[b - 1] != cls:
            continue
        r0 = int(np.searchsorted(ts, cls, side="left"))
        r1 = int(np.searchsorted(ts, cls, side="right"))
        # rows of this class inside strip s miss cols [r0, b)
        rows = np.arange(b, r1)
        cols = np.arange(r0, b)
        sblk = (qf[rows] @ qf[cols].T) / (SCALE * SCALE)
        eblk = np.exp(sblk / TAU).astype(ml_dtypes.bfloat16).astype(np.float64)
        rsem[rows] += eblk.sum(1)

    p = np.bincount(ts)[ts].astype(np.float64)
    neg = rse - rsem
    s2 = rsem - eii
    g = np.zeros((int(ts.max()) + 1, D), np.float64)
    np.add.at(g, ts, fns.astype(np.float64))
    dotg = (fns.astype(np.float64) * g[ts]).sum(1)
    numer = (p - 1.0) * np.log(neg) + s2 / neg - (dotg - 1.0) / TAU
    loss = (numer / p).sum() / p.sum()
    return np.float32(loss)


def _run(in_maps, trace=False):
    from concourse.bass_utils import run_bass_kernel_spmd
    nc = _get_nc()
    return run_bass_kernel_spmd(
        nc, in_maps, core_ids=list(range(NCORES)), trace=trace,
    )


def kernel(features, targets):
    fns, ts, q, in_maps = _host_prep(features, targets)
    res = _run(in_maps, trace=False)
    return _host_post(fns, ts, q, res.results)
